# revision 13
# baseline (speedup 1.0000x reference)
"""Self-contained TP-over-heads DeepseekAttention kernel for 8 TRN2 cores.

Sharding: tensor-parallel across heads (4 heads/core). Each core computes
Q/K/V projections for its heads (bf16 matmuls), RoPE, attention with
transposed-scores layout (fp32r matmuls), a row-parallel partial o_proj,
then a ReduceScatter(add) over the sequence dim. Host concatenates the
8 per-core [256, 4096] slices.
"""

import numpy as np
import ml_dtypes

import concourse.bass as bass
import concourse.mybir as mybir
import concourse.tile as tile
from concourse import bacc
from concourse.bass_utils import run_bass_kernel_spmd

# problem shapes (hardcoded per contract)
S = 2048
H = 4096
NH = 32
D = 128
NC = 8
HPC = NH // NC          # 4 heads per core
DPC = HPC * D           # 512 head-dims per core
KT = H // 128           # 32 contraction tiles over hidden
SCH = 512               # s-chunk for projections
NSC = S // SCH          # 4
ST = S // 128           # 16 s-tiles
QCH = 512               # q-chunk in attention
NQC = S // QCH          # 4
NKT = S // 128          # 16 k-tiles in attention
SPC = S // NC           # 256 rows of output per core

f32 = mybir.dt.float32
f32r = mybir.dt.float32r
bf16 = mybir.dt.bfloat16
bf16_np = ml_dtypes.bfloat16

ROPE_THETA = 10000.0
SCALE = float(1.0 / np.sqrt(D))

_CACHE: dict = {}


def _build():
    nc = bacc.Bacc("TRN2", target_bir_lowering=False, debug=False, num_devices=NC)

    # ---- I/O ----
    xt = nc.dram_tensor("xt", [KT, 128, S], bf16, kind="ExternalInput").ap()
    wq = nc.dram_tensor("wq", [KT, 128, DPC], bf16, kind="ExternalInput").ap()
    wk = nc.dram_tensor("wk", [KT, 128, DPC], bf16, kind="ExternalInput").ap()
    wv = nc.dram_tensor("wv", [KT, 128, DPC], bf16, kind="ExternalInput").ap()
    wo = nc.dram_tensor("wo", [HPC, 128, H], f32r, kind="ExternalInput").ap()
    cost = nc.dram_tensor("cost", [128, S], f32, kind="ExternalInput").ap()
    sint = nc.dram_tensor("sint", [128, S], f32, kind="ExternalInput").ap()
    rmat = nc.dram_tensor("rmat", [128, 128], f32r, kind="ExternalInput").ap()
    ones_col = nc.dram_tensor("ones_col", [128, 1], f32r, kind="ExternalInput").ap()
    ones_row = nc.dram_tensor("ones_row", [1, 128], f32r, kind="ExternalInput").ap()
    out_ext = nc.dram_tensor("out", [SPC, H], f32, kind="ExternalOutput").ap()

    from contextlib import ExitStack

    with tile.TileContext(nc) as tc:
        with tc.tile_pool(name="dram", bufs=1, space="DRAM") as dram_pool:
            qkrope = dram_pool.tile([2 * HPC, 128, S], f32r,
                                    name="qkrope")  # [q0..q3, k0..k3]
            partial = dram_pool.tile([S, H], f32, name="partial")
            rs_out = dram_pool.tile([SPC, H], f32, name="rs_out")

            # ============ Phase 1: Q/K projections + RoPE (spill to DRAM) ===
            with (
                tc.tile_pool(name="wqk", bufs=1) as wqk_pool,
                tc.tile_pool(name="xt1", bufs=2) as xt1_pool,
                tc.tile_pool(name="ropec", bufs=1) as rope_pool,
                tc.tile_pool(name="qktmp", bufs=3) as qktmp_pool,
                tc.tile_pool(name="psA", bufs=2, space="PSUM") as psA,
            ):
                cos_sb = rope_pool.tile([128, S], f32, tag="cos")
                sin_sb = rope_pool.tile([128, S], f32, tag="sin")
                rm_sb = rope_pool.tile([128, 128], f32r, tag="rm")
                nc.sync.dma_start(cos_sb[:], cost[:])
                nc.sync.dma_start(sin_sb[:], sint[:])
                nc.sync.dma_start(rm_sb[:], rmat[:])
                wq_sb = wqk_pool.tile([128, KT, DPC], bf16, tag="wq")
                wk_sb = wqk_pool.tile([128, KT, DPC], bf16, tag="wk")
                nc.sync.dma_start(wq_sb[:], wq.rearrange("k p n -> p k n"))
                nc.sync.dma_start(wk_sb[:], wk.rearrange("k p n -> p k n"))

                for sc in range(NSC):
                    s0 = sc * SCH
                    x_sb = xt1_pool.tile([128, KT, SCH], bf16, tag="x")
                    nc.sync.dma_start(
                        x_sb[:], xt.rearrange("k p s -> p k s")[:, :, s0:s0 + SCH]
                    )
                    for pi, w_sb in ((0, wq_sb), (1, wk_sb)):
                        for h in range(HPC):
                            ps = psA.tile([128, SCH], f32, tag="proj")
                            for kt in range(KT):
                                nc.tensor.matmul(
                                    ps[:],
                                    w_sb[:, kt, h * 128:(h + 1) * 128],
                                    x_sb[:, kt, :],
                                    start=(kt == 0),
                                    stop=(kt == KT - 1),
                                )
                            raw = qktmp_pool.tile([128, SCH], f32r, tag="raw")
                            nc.scalar.copy(raw[:], ps[:])
                            psr = psA.tile([128, SCH], f32, tag="rot")
                            nc.tensor.matmul(psr[:], rm_sb[:], raw[:],
                                             start=True, stop=True)
                            t1 = qktmp_pool.tile([128, SCH], f32, tag="t1")
                            nc.vector.tensor_mul(t1[:], raw[:], cos_sb[:, s0:s0 + SCH])
                            t2 = qktmp_pool.tile([128, SCH], f32, tag="t2")
                            nc.vector.tensor_mul(t2[:], psr[:], sin_sb[:, s0:s0 + SCH])
                            rope_t = qktmp_pool.tile([128, SCH], f32r, tag="rope")
                            nc.vector.tensor_add(rope_t[:], t1[:], t2[:])
                            nc.sync.dma_start(
                                qkrope[pi * HPC + h, :, s0:s0 + SCH], rope_t[:]
                            )

            # ============ Phase 2: V projection (natural [s, d] layout) ======
            ot_store_ctx = ExitStack()
            ot_store = ot_store_ctx.enter_context(tc.tile_pool(name="otstore", bufs=1))
            v_store_ctx = ExitStack()
            v_store = v_store_ctx.enter_context(tc.tile_pool(name="vstore", bufs=1))
            with (
                tc.tile_pool(name="wv", bufs=1) as wv_pool,
                tc.tile_pool(name="xt2", bufs=3) as xt2_pool,
                tc.tile_pool(name="psB", bufs=2, space="PSUM") as psB,
            ):
                wv_sb = wv_pool.tile([128, KT, DPC], bf16, tag="wv")
                nc.sync.dma_start(wv_sb[:], wv.rearrange("k p n -> p k n"))
                v_tiles = []
                for st in range(ST):
                    x_sb = xt2_pool.tile([128, KT, 128], bf16, tag="x2")
                    nc.sync.dma_start(
                        x_sb[:],
                        xt.rearrange("k p s -> p k s")[:, :, st * 128:(st + 1) * 128],
                    )
                    ps = psB.tile([128, DPC], f32, tag="vp")
                    for kt in range(KT):
                        nc.tensor.matmul(
                            ps[:], x_sb[:, kt, :], wv_sb[:, kt, :],
                            start=(kt == 0), stop=(kt == KT - 1),
                        )
                    v_t = v_store.tile([128, DPC], f32r, tag=f"v{st}")
                    nc.scalar.copy(v_t[:], ps[:])
                    v_tiles.append(v_t)

            ot_tiles = [ot_store.tile([128, S], f32r, tag=f"ot{h}",
                                      name=f"ot{h}")
                        for h in range(HPC)]

            # ============ Phase 3: attention (transposed scores) =============
            with (
                tc.tile_pool(name="ksb", bufs=2) as k_pool,
                tc.tile_pool(name="qsb", bufs=3) as q_pool,
                tc.tile_pool(name="pt", bufs=NKT + 4) as pt_pool,
                tc.tile_pool(name="tree", bufs=12) as tree_pool,
                tc.tile_pool(name="attnmisc", bufs=3) as misc_pool,
                tc.tile_pool(name="psC", bufs=2, space="PSUM") as psC,
            ):
                for h in range(HPC):
                    k_sb = k_pool.tile([128, S], f32r, tag="k")
                    nc.sync.dma_start(k_sb[:], qkrope[HPC + h])
                    for qc in range(NQC):
                        q0 = qc * QCH
                        q_sb = q_pool.tile([128, QCH], f32r, tag="q")
                        nc.sync.dma_start(q_sb[:], qkrope[h, :, q0:q0 + QCH])

                        # scores^T tiles + exp
                        pts = []
                        for kt in range(NKT):
                            ps_s = psC.tile([128, QCH], f32, tag="scores")
                            nc.tensor.matmul(
                                ps_s[:],
                                k_sb[:, kt * 128:(kt + 1) * 128],
                                q_sb[:],
                                start=True, stop=True,
                            )
                            pt = pt_pool.tile([128, QCH], f32r, tag="pt")
                            nc.scalar.activation(
                                pt[:], ps_s[:],
                                mybir.ActivationFunctionType.Exp, scale=SCALE,
                            )
                            pts.append(pt)

                        # attn @ V accumulation (out^T layout)
                        ps_o = psC.tile([128, QCH], f32, tag="vmm")
                        for kt in range(NKT):
                            nc.tensor.matmul(
                                ps_o[:],
                                v_tiles[kt][:, h * 128:(h + 1) * 128],
                                pts[kt][:],
                                start=(kt == 0), stop=(kt == NKT - 1),
                            )

                        # denominator: binary tree sum of the 16 P^T tiles
                        lvl = pts
                        while len(lvl) > 1:
                            nxt = []
                            for i in range(0, len(lvl) - 1, 2):
                                dt_ = f32r if len(lvl) == 2 else f32
                                tsum = tree_pool.tile([128, QCH], dt_, tag="tr")
                                nc.vector.tensor_add(tsum[:], lvl[i][:], lvl[i + 1][:])
                                nxt.append(tsum)
                            if len(lvl) % 2 == 1:
                                nxt.append(lvl[-1])
                            lvl = nxt
                        t_sum = lvl[0]  # [128, QCH] f32r

                        # cross-partition sum -> [1, QCH]; broadcast + recip
                        oc_sb = misc_pool.tile([128, 1], f32r, tag="ones_c")
                        if h == 0 and qc == 0:
                            nc.sync.dma_start(oc_sb[:], ones_col[:])
                            or_sb = misc_pool.tile([1, 128], f32r, tag="ones_r")
                            nc.sync.dma_start(or_sb[:], ones_row[:])
                            _ones = (oc_sb, or_sb)
                        oc_sb, or_sb = _ones
                        ps_sum = psC.tile([1, QCH], f32, tag="sum")
                        nc.tensor.matmul(ps_sum[:], oc_sb[:], t_sum[:],
                                         start=True, stop=True)
                        sum_sb = misc_pool.tile([1, QCH], f32r, tag="sum_sb")
                        nc.scalar.copy(sum_sb[:], ps_sum[:])
                        ps_bc = psC.tile([128, QCH], f32, tag="bc")
                        nc.tensor.matmul(ps_bc[:], or_sb[:], sum_sb[:],
                                         start=True, stop=True)
                        recip_sb = misc_pool.tile([128, QCH], f32, tag="recip")
                        nc.vector.reciprocal(recip_sb[:], ps_bc[:])

                        # normalize: out^T = psum_vmm * recip
                        nc.vector.tensor_mul(
                            ot_tiles[h][:, q0:q0 + QCH], ps_o[:], recip_sb[:]
                        )

            v_store_ctx.close()

            # ============ Phase 4: o_proj (row-parallel partial) =============
            with (
                tc.tile_pool(name="wo", bufs=1) as wo_pool,
                tc.tile_pool(name="drain", bufs=4) as drain_pool,
                tc.tile_pool(name="psD", bufs=4, space="PSUM") as psD,
            ):
                wo_sb = wo_pool.tile([128, HPC, H], f32r, tag="wo")
                nc.sync.dma_start(wo_sb[:], wo.rearrange("h p n -> p h n"))
                for nci in range(H // 512):
                    n0 = nci * 512
                    for qt in range(ST):
                        ps = psD.tile([128, 512], f32, tag="op")
                        for h in range(HPC):
                            nc.tensor.matmul(
                                ps[:],
                                ot_tiles[h][:, qt * 128:(qt + 1) * 128],
                                wo_sb[:, h, n0:n0 + 512],
                                start=(h == 0), stop=(h == HPC - 1),
                            )
                        dr = drain_pool.tile([128, 512], f32, tag="dr")
                        nc.scalar.copy(dr[:], ps[:])
                        nc.sync.dma_start(
                            partial[qt * 128:(qt + 1) * 128, n0:n0 + 512], dr[:]
                        )

            ot_store_ctx.close()

            # ============ Phase 5: ReduceScatter over seq + output ===========
            nc.gpsimd.collective_compute(
                "ReduceScatter",
                mybir.AluOpType.add,
                replica_groups=[list(range(NC))],
                ins=[partial.opt()],
                outs=[rs_out.opt()],
            )
            nc.gpsimd.dma_start(out_ext[:], rs_out[:])

    nc.compile()
    return nc


def _host_prep(positions, hidden_states, Wq, Wk, Wv, Wo):
    X = np.asarray(hidden_states, dtype=np.float32).reshape(S, H)
    XT = np.ascontiguousarray(X.T).astype(bf16_np).reshape(KT, 128, S)

    pos = np.asarray(positions).astype(np.float32)
    inv_freq = (1.0 / (ROPE_THETA ** (np.arange(0, D, 2, dtype=np.float32) / D)))
    freqs = pos[:, None] * inv_freq[None, :]
    emb = np.concatenate([freqs, freqs], axis=-1)        # [S, D]
    cosT = np.ascontiguousarray(np.cos(emb).astype(np.float32).T)  # [128, S]
    sinT = np.ascontiguousarray(np.sin(emb).astype(np.float32).T)

    rm = np.zeros((128, 128), np.float32)
    idx = np.arange(64)
    rm[64 + idx, idx] = -1.0   # out[0:64]  = -in[64:128]
    rm[idx, 64 + idx] = 1.0    # out[64:128] = in[0:64]

    Wq = np.asarray(Wq, dtype=np.float32)
    Wk = np.asarray(Wk, dtype=np.float32)
    Wv = np.asarray(Wv, dtype=np.float32)
    Wo = np.asarray(Wo, dtype=np.float32)

    in_maps = []
    for c in range(NC):
        sl = slice(DPC * c, DPC * (c + 1))
        wq_c = np.ascontiguousarray(Wq[sl, :].T).astype(bf16_np).reshape(KT, 128, DPC)
        wk_c = np.ascontiguousarray(Wk[sl, :].T).astype(bf16_np).reshape(KT, 128, DPC)
        wv_c = np.ascontiguousarray(Wv[sl, :].T).astype(bf16_np).reshape(KT, 128, DPC)
        wo_c = np.ascontiguousarray(Wo[:, sl].T).reshape(HPC, 128, H)
        in_maps.append({
            "xt": XT, "wq": wq_c, "wk": wk_c, "wv": wv_c, "wo": wo_c,
            "cost": cosT, "sint": sinT, "rmat": rm,
            "ones_col": np.ones((128, 1), np.float32),
            "ones_row": np.ones((1, 128), np.float32),
        })
    return in_maps


def kernel(positions, hidden_states, Wq, Wk, Wv, Wo):
    if "nc" not in _CACHE:
        _CACHE["nc"] = _build()
    nc = _CACHE["nc"]
    in_maps = _host_prep(positions, hidden_states, Wq, Wk, Wv, Wo)
    res = run_bass_kernel_spmd(nc, in_maps, list(range(NC)))
    out = np.concatenate([res.results[c]["out"] for c in range(NC)], axis=0)
    return out.reshape(1, S, H).astype(np.float32)
